# revision 12
# baseline (speedup 1.0000x reference)
"""Batched RX-gate application: out = state @ (cos(t/2) I - i sin(t/2) X_q).

X_q = kron(I_32, X, I_64) is the Pauli-X permutation flipping bit 6 of the
column index (j ^ 64).  With state = re + i*im and f = flip(j ^ 64):
    out_re[:, j] = c*re[:, j] + s*im[:, j^64]
    out_im[:, j] = c*im[:, j] - s*re[:, j^64]
where c = cos(theta/2), s = sin(theta/2).

Factored as two DVE ops per output, in place (stable for any theta):
    o_re = c*re            (tensor_scalar, 2x perf mode)
    o_re = (im_f*s) + o_re (scalar_tensor_tensor, 1x)
    o_im = c*im
    o_im = (re_f*-s) + o_im
The tensor_scalar ops are issued first so they absorb every cross-engine
sync wait (DMA sems, slot WAR); the STTs then need no waits at all —
walrus's STT encoding has too few sync-wait slots for more.

Sharding: batch rows (4096) split 512/core across 8 NeuronCores; the
gate coefficients are replicated.  No communication.
"""

import sys

if "/opt/trn_rl_repo" not in sys.path:
    sys.path.insert(0, "/opt/trn_rl_repo")

import numpy as np

import concourse.bacc as bacc
import concourse.bass as bass
import concourse.mybir as mybir
from concourse import bass_utils
from concourse.tile import TileContext

N_CORES = 8
BATCH = 4096
N = 4096
ROWS = BATCH // N_CORES  # rows per core
P = 128                  # SBUF partitions
FLIP = 64                # column flip: j ^ 64
BLK = 2 * FLIP           # 128-wide column blocks; flip swaps halves

F32 = mybir.dt.float32


def _build_nc(rows: int = ROWS) -> bass.Bass:
    """Per-core Bass module."""
    nc = bacc.Bacc("TRN2", target_bir_lowering=False, debug=False)
    sr = nc.dram_tensor("sr", [rows, N], F32, kind="ExternalInput").ap()
    si = nc.dram_tensor("si", [rows, N], F32, kind="ExternalInput").ap()
    cf = nc.dram_tensor("cf", [P, 4], F32, kind="ExternalInput").ap()
    dst_re = nc.dram_tensor("out_re", [rows, N], F32, kind="ExternalOutput").ap()
    dst_im = nc.dram_tensor("out_im", [rows, N], F32, kind="ExternalOutput").ap()

    mult = mybir.AluOpType.mult
    add = mybir.AluOpType.add
    lo = slice(0, FLIP)
    hi = slice(FLIP, BLK)

    with TileContext(nc) as tc:
        with (
            tc.tile_pool(name="coef", bufs=1) as cpool,
            tc.tile_pool(name="in", bufs=3) as ipool,
            tc.tile_pool(name="out", bufs=2) as opool,
        ):
            coef = cpool.tile([P, 4], F32, name="coef")
            warm = cpool.tile([P, 4], F32, name="warm")
            # tiny first DMAs arm both HWDGE rings so the first real loads
            # don't eat the ring's first-byte/descriptor-gen latency
            nc.sync.dma_start(out=coef[:, :], in_=cf)
            nc.scalar.dma_start(out=warm[:, :], in_=cf)
            c_ap = coef[:, 0:1]     # cos(theta/2)
            s_ap = coef[:, 1:2]     # sin(theta/2)
            negs_ap = coef[:, 2:3]  # -sin(theta/2)

            ts = nc.vector.tensor_scalar
            stt = nc.vector.scalar_tensor_tensor
            for i in range(rows // P):
                sl = slice(i * P, (i + 1) * P)
                t_re = ipool.tile([P, N], F32, name="t_re", tag="t_re")
                t_im = ipool.tile([P, N], F32, name="t_im", tag="t_im")
                o_re = opool.tile([P, N], F32, name="o_re", tag="o_re")
                o_im = opool.tile([P, N], F32, name="o_im", tag="o_im")
                # alternate every transfer across the two HWDGE rings so
                # each ring's per-DMA overhead hides under the other's data
                nc.sync.dma_start(out=t_re[:, :], in_=sr[sl, :])
                nc.scalar.dma_start(out=t_im[:, :], in_=si[sl, :])

                re3 = t_re[:, :].rearrange("p (b c) -> p b c", c=BLK)
                im3 = t_im[:, :].rearrange("p (b c) -> p b c", c=BLK)
                ore = o_re[:, :].rearrange("p (b c) -> p b c", c=BLK)
                oim = o_im[:, :].rearrange("p (b c) -> p b c", c=BLK)

                # tensor_scalar first: these take the DMA-sem + slot-WAR
                # waits, so the STTs below issue with no sync waits (the
                # STT walrus encoding supports very few).
                ts(o_re[:, :], t_re[:, :], c_ap, None, mult)   # o_re = c*re
                ts(o_im[:, :], t_im[:, :], c_ap, None, mult)   # o_im = c*im
                # o_re += s*im_f ; o_im += -s*re_f  (in place, flipped AP)
                stt(ore[:, :, lo], im3[:, :, hi], s_ap, ore[:, :, lo], mult, add)
                stt(ore[:, :, hi], im3[:, :, lo], s_ap, ore[:, :, hi], mult, add)
                stt(oim[:, :, lo], re3[:, :, hi], negs_ap, oim[:, :, lo], mult, add)
                stt(oim[:, :, hi], re3[:, :, lo], negs_ap, oim[:, :, hi], mult, add)

                nc.sync.dma_start(out=dst_re[sl, :], in_=o_re[:, :])
                nc.scalar.dma_start(out=dst_im[sl, :], in_=o_im[:, :])
    nc.compile()
    return nc


_NC_CACHE: dict = {}


def _get_nc() -> bass.Bass:
    if "nc" not in _NC_CACHE:
        _NC_CACHE["nc"] = _build_nc(ROWS)
    return _NC_CACHE["nc"]


def _coef_array(theta: float) -> np.ndarray:
    c = np.cos(theta / 2.0)
    s = np.sin(theta / 2.0)
    coef = np.zeros((P, 4), np.float32)
    coef[:, 0] = c
    coef[:, 1] = s
    coef[:, 2] = -s
    return coef


def _run(state_re, state_im, theta, **spmd_kwargs):
    theta = float(np.asarray(theta))
    coef = _coef_array(theta)
    nc = _get_nc()
    sr = np.ascontiguousarray(np.asarray(state_re, dtype=np.float32))
    si = np.ascontiguousarray(np.asarray(state_im, dtype=np.float32))
    in_maps = [
        {
            "sr": sr[c * ROWS : (c + 1) * ROWS],
            "si": si[c * ROWS : (c + 1) * ROWS],
            "cf": coef,
        }
        for c in range(N_CORES)
    ]
    res = bass_utils.run_bass_kernel_spmd(
        nc, in_maps, core_ids=list(range(N_CORES)), **spmd_kwargs
    )
    out_re = np.concatenate([res.results[c]["out_re"] for c in range(N_CORES)], axis=0)
    out_im = np.concatenate([res.results[c]["out_im"] for c in range(N_CORES)], axis=0)
    return (out_re, out_im), res


def kernel(state_re, state_im, theta):
    (out_re, out_im), _ = _run(state_re, state_im, theta)
    return out_re, out_im


# revision 17
# speedup vs baseline: 1.0460x; 1.0460x over previous
"""Batched RX-gate application: out = state @ (cos(t/2) I - i sin(t/2) X_q).

X_q = kron(I_32, X, I_64) is the Pauli-X permutation flipping bit 6 of the
column index (j ^ 64).  With state = re + i*im and f = flip(j ^ 64):
    out_re[:, j] = c*re[:, j] + s*im[:, j^64]
    out_im[:, j] = c*im[:, j] - s*re[:, j^64]
where c = cos(theta/2), s = sin(theta/2).

Factored as two DVE ops per output, in place (stable for any theta):
    o_re = c*re            (tensor_scalar, 2x perf mode)
    o_re = (im_f*s) + o_re (scalar_tensor_tensor, 1x)
    o_im = c*im
    o_im = (re_f*-s) + o_im
The tensor_scalar ops are issued first so they absorb every cross-engine
sync wait (DMA sems, slot WAR); the STTs then need no waits at all —
walrus's STT encoding has too few sync-wait slots for more.

Sharding: batch rows (4096) split 512/core across 8 NeuronCores; the
gate coefficients are replicated.  No communication.
"""

import contextlib
import os
import sys

if "/opt/trn_rl_repo" not in sys.path:
    sys.path.insert(0, "/opt/trn_rl_repo")

import numpy as np

import concourse.bacc as bacc
import concourse.bass as bass
import concourse.mybir as mybir
from concourse import bass_utils
from concourse.tile import TileContext

N_CORES = 8
BATCH = 4096
N = 4096
ROWS = BATCH // N_CORES  # rows per core
P = 128                  # SBUF partitions
FLIP = 64                # column flip: j ^ 64
BLK = 2 * FLIP           # 128-wide column blocks; flip swaps halves

F32 = mybir.dt.float32


def _build_nc(rows: int = ROWS) -> bass.Bass:
    """Per-core Bass module."""
    nc = bacc.Bacc("TRN2", target_bir_lowering=False, debug=False)
    sr = nc.dram_tensor("sr", [rows, N], F32, kind="ExternalInput").ap()
    si = nc.dram_tensor("si", [rows, N], F32, kind="ExternalInput").ap()
    cf = nc.dram_tensor("cf", [P, 4], F32, kind="ExternalInput").ap()
    dst_re = nc.dram_tensor("out_re", [rows, N], F32, kind="ExternalOutput").ap()
    dst_im = nc.dram_tensor("out_im", [rows, N], F32, kind="ExternalOutput").ap()

    mult = mybir.AluOpType.mult
    add = mybir.AluOpType.add
    lo = slice(0, FLIP)
    hi = slice(FLIP, BLK)

    with TileContext(nc) as tc:
        with (
            tc.tile_pool(name="coef", bufs=1) as cpool,
            tc.tile_pool(name="in", bufs=3) as ipool,
            tc.tile_pool(name="out", bufs=2) as opool,
        ):
            coef = cpool.tile([P, 4], F32, name="coef")
            nc.sync.dma_start(out=coef[:, :], in_=cf)
            c_ap = coef[:, 0:1]     # cos(theta/2)
            s_ap = coef[:, 1:2]     # sin(theta/2)
            negs_ap = coef[:, 2:3]  # -sin(theta/2)

            ts = nc.vector.tensor_scalar
            stt = nc.vector.scalar_tensor_tensor
            for i in range(rows // P):
                sl = slice(i * P, (i + 1) * P)
                t_re = ipool.tile([P, N], F32, name="t_re", tag="t_re")
                t_im = ipool.tile([P, N], F32, name="t_im", tag="t_im")
                o_re = opool.tile([P, N], F32, name="o_re", tag="o_re")
                o_im = opool.tile([P, N], F32, name="o_im", tag="o_im")
                # loads on the SP HWDGE ring, stores on the ACT ring:
                # separate FIFO rings overlap their per-DMA overheads
                nc.sync.dma_start(out=t_re[:, :], in_=sr[sl, :])
                nc.sync.dma_start(out=t_im[:, :], in_=si[sl, :])

                re3 = t_re[:, :].rearrange("p (b c) -> p b c", c=BLK)
                im3 = t_im[:, :].rearrange("p (b c) -> p b c", c=BLK)
                ore = o_re[:, :].rearrange("p (b c) -> p b c", c=BLK)
                oim = o_im[:, :].rearrange("p (b c) -> p b c", c=BLK)

                # tensor_scalar first: these take the DMA-sem + slot-WAR
                # waits, so the STTs below issue with no sync waits (the
                # STT walrus encoding supports very few).
                ts(o_re[:, :], t_re[:, :], c_ap, None, mult)   # o_re = c*re
                ts(o_im[:, :], t_im[:, :], c_ap, None, mult)   # o_im = c*im
                # o_re += s*im_f ; o_im += -s*re_f  (in place, flipped AP)
                stt(ore[:, :, lo], im3[:, :, hi], s_ap, ore[:, :, lo], mult, add)
                stt(ore[:, :, hi], im3[:, :, lo], s_ap, ore[:, :, hi], mult, add)
                stt(oim[:, :, lo], re3[:, :, hi], negs_ap, oim[:, :, lo], mult, add)
                stt(oim[:, :, hi], re3[:, :, lo], negs_ap, oim[:, :, hi], mult, add)

                nc.scalar.dma_start(out=dst_re[sl, :], in_=o_re[:, :])
                nc.gpsimd.dma_start(out=dst_im[sl, :], in_=o_im[:, :])
    nc.compile()
    return nc


_NC_CACHE: dict = {}


def _get_nc() -> bass.Bass:
    if "nc" not in _NC_CACHE:
        _NC_CACHE["nc"] = _build_nc(ROWS)
    return _NC_CACHE["nc"]


def _coef_array(theta: float) -> np.ndarray:
    c = np.cos(theta / 2.0)
    s = np.sin(theta / 2.0)
    coef = np.zeros((P, 4), np.float32)
    coef[:, 0] = c
    coef[:, 1] = s
    coef[:, 2] = -s
    return coef


@contextlib.contextmanager
def _force_no_trace():
    """Tracing needs antenv.axon_hooks (absent in some images); make sure a
    stray BASS_TRACE env var can't push us onto that path."""
    old = os.environ.get("BASS_NEVER_TRACE")
    os.environ["BASS_NEVER_TRACE"] = "1"
    try:
        yield
    finally:
        if old is None:
            os.environ.pop("BASS_NEVER_TRACE", None)
        else:
            os.environ["BASS_NEVER_TRACE"] = old


def _run(state_re, state_im, theta, **spmd_kwargs):
    theta = float(np.asarray(theta))
    coef = _coef_array(theta)
    nc = _get_nc()
    sr = np.ascontiguousarray(np.asarray(state_re, dtype=np.float32))
    si = np.ascontiguousarray(np.asarray(state_im, dtype=np.float32))
    in_maps = [
        {
            "sr": sr[c * ROWS : (c + 1) * ROWS],
            "si": si[c * ROWS : (c + 1) * ROWS],
            "cf": coef,
        }
        for c in range(N_CORES)
    ]
    guard = contextlib.nullcontext() if spmd_kwargs.get("trace") else _force_no_trace()
    with guard:
        res = bass_utils.run_bass_kernel_spmd(
            nc, in_maps, core_ids=list(range(N_CORES)), **spmd_kwargs
        )
    out_re = np.concatenate([res.results[c]["out_re"] for c in range(N_CORES)], axis=0)
    out_im = np.concatenate([res.results[c]["out_im"] for c in range(N_CORES)], axis=0)
    return (out_re, out_im), res


def kernel(state_re, state_im, theta):
    (out_re, out_im), _ = _run(state_re, state_im, theta)
    return out_re, out_im


# revision 18
# speedup vs baseline: 1.0903x; 1.0423x over previous
"""Batched RX-gate application: out = state @ (cos(t/2) I - i sin(t/2) X_q).

X_q = kron(I_32, X, I_64) is the Pauli-X permutation flipping bit 6 of the
column index (j ^ 64).  With state = re + i*im and f = flip(j ^ 64):
    out_re[:, j] = c*re[:, j] + s*im[:, j^64]
    out_im[:, j] = c*im[:, j] - s*re[:, j^64]
where c = cos(theta/2), s = sin(theta/2).

Factored as two DVE ops per output, in place (stable for any theta):
    o_re = c*re            (tensor_scalar, 2x perf mode)
    o_re = (im_f*s) + o_re (scalar_tensor_tensor, 1x)
    o_im = c*im
    o_im = (re_f*-s) + o_im
The tensor_scalar ops are issued first so they absorb every cross-engine
sync wait (DMA sems, slot WAR); the STTs then need no waits at all —
walrus's STT encoding has too few sync-wait slots for more.

Sharding: batch rows (4096) split 512/core across 8 NeuronCores; the
gate coefficients are replicated.  No communication.
"""

import contextlib
import os
import sys

if "/opt/trn_rl_repo" not in sys.path:
    sys.path.insert(0, "/opt/trn_rl_repo")

import numpy as np

import concourse.bacc as bacc
import concourse.bass as bass
import concourse.mybir as mybir
from concourse import bass_utils
from concourse.tile import TileContext

N_CORES = 8
BATCH = 4096
N = 4096
ROWS = BATCH // N_CORES  # rows per core
P = 128                  # SBUF partitions
FLIP = 64                # column flip: j ^ 64
BLK = 2 * FLIP           # 128-wide column blocks; flip swaps halves

F32 = mybir.dt.float32


def _build_nc(rows: int = ROWS) -> bass.Bass:
    """Per-core Bass module."""
    nc = bacc.Bacc("TRN2", target_bir_lowering=False, debug=False)
    sr = nc.dram_tensor("sr", [rows, N], F32, kind="ExternalInput").ap()
    si = nc.dram_tensor("si", [rows, N], F32, kind="ExternalInput").ap()
    cf = nc.dram_tensor("cf", [P, 4], F32, kind="ExternalInput").ap()
    dst_re = nc.dram_tensor("out_re", [rows, N], F32, kind="ExternalOutput").ap()
    dst_im = nc.dram_tensor("out_im", [rows, N], F32, kind="ExternalOutput").ap()

    mult = mybir.AluOpType.mult
    add = mybir.AluOpType.add
    lo = slice(0, FLIP)
    hi = slice(FLIP, BLK)

    with TileContext(nc) as tc:
        with (
            tc.tile_pool(name="coef", bufs=1) as cpool,
            tc.tile_pool(name="in", bufs=3) as ipool,
            tc.tile_pool(name="out", bufs=2) as opool,
        ):
            coef = cpool.tile([P, 4], F32, name="coef")
            nc.sync.dma_start(out=coef[:, :], in_=cf)
            c_ap = coef[:, 0:1]     # cos(theta/2)
            s_ap = coef[:, 1:2]     # sin(theta/2)
            negs_ap = coef[:, 2:3]  # -sin(theta/2)

            ts = nc.vector.tensor_scalar
            stt = nc.vector.scalar_tensor_tensor
            for i in range(rows // P):
                sl = slice(i * P, (i + 1) * P)
                t_re = ipool.tile([P, N], F32, name="t_re", tag="t_re")
                t_im = ipool.tile([P, N], F32, name="t_im", tag="t_im")
                o_re = opool.tile([P, N], F32, name="o_re", tag="o_re")
                o_im = opool.tile([P, N], F32, name="o_im", tag="o_im")
                # loads on the SP HWDGE ring, stores split across the ACT
                # HWDGE ring and SWDGE: separate streams overlap their
                # per-DMA overheads.  Chunk 0 loads go via SWDGE (shorter
                # first-byte latency) to shrink the pipeline-fill holes.
                ld = nc.gpsimd if i == 0 else nc.sync
                ld.dma_start(out=t_re[:, :], in_=sr[sl, :])
                ld.dma_start(out=t_im[:, :], in_=si[sl, :])

                re3 = t_re[:, :].rearrange("p (b c) -> p b c", c=BLK)
                im3 = t_im[:, :].rearrange("p (b c) -> p b c", c=BLK)
                ore = o_re[:, :].rearrange("p (b c) -> p b c", c=BLK)
                oim = o_im[:, :].rearrange("p (b c) -> p b c", c=BLK)

                # tensor_scalar first: these take the DMA-sem + slot-WAR
                # waits, so the STTs below issue with no sync waits (the
                # STT walrus encoding supports very few).
                ts(o_re[:, :], t_re[:, :], c_ap, None, mult)   # o_re = c*re
                ts(o_im[:, :], t_im[:, :], c_ap, None, mult)   # o_im = c*im
                # o_re += s*im_f ; o_im += -s*re_f  (in place, flipped AP)
                stt(ore[:, :, lo], im3[:, :, hi], s_ap, ore[:, :, lo], mult, add)
                stt(ore[:, :, hi], im3[:, :, lo], s_ap, ore[:, :, hi], mult, add)
                stt(oim[:, :, lo], re3[:, :, hi], negs_ap, oim[:, :, lo], mult, add)
                stt(oim[:, :, hi], re3[:, :, lo], negs_ap, oim[:, :, hi], mult, add)

                nc.scalar.dma_start(out=dst_re[sl, :], in_=o_re[:, :])
                nc.gpsimd.dma_start(out=dst_im[sl, :], in_=o_im[:, :])
    nc.compile()
    return nc


_NC_CACHE: dict = {}


def _get_nc() -> bass.Bass:
    if "nc" not in _NC_CACHE:
        _NC_CACHE["nc"] = _build_nc(ROWS)
    return _NC_CACHE["nc"]


def _coef_array(theta: float) -> np.ndarray:
    c = np.cos(theta / 2.0)
    s = np.sin(theta / 2.0)
    coef = np.zeros((P, 4), np.float32)
    coef[:, 0] = c
    coef[:, 1] = s
    coef[:, 2] = -s
    return coef


@contextlib.contextmanager
def _force_no_trace():
    """Tracing needs antenv.axon_hooks (absent in some images); make sure a
    stray BASS_TRACE env var can't push us onto that path."""
    old = os.environ.get("BASS_NEVER_TRACE")
    os.environ["BASS_NEVER_TRACE"] = "1"
    try:
        yield
    finally:
        if old is None:
            os.environ.pop("BASS_NEVER_TRACE", None)
        else:
            os.environ["BASS_NEVER_TRACE"] = old


def _run(state_re, state_im, theta, **spmd_kwargs):
    theta = float(np.asarray(theta))
    coef = _coef_array(theta)
    nc = _get_nc()
    sr = np.ascontiguousarray(np.asarray(state_re, dtype=np.float32))
    si = np.ascontiguousarray(np.asarray(state_im, dtype=np.float32))
    in_maps = [
        {
            "sr": sr[c * ROWS : (c + 1) * ROWS],
            "si": si[c * ROWS : (c + 1) * ROWS],
            "cf": coef,
        }
        for c in range(N_CORES)
    ]
    guard = contextlib.nullcontext() if spmd_kwargs.get("trace") else _force_no_trace()
    with guard:
        res = bass_utils.run_bass_kernel_spmd(
            nc, in_maps, core_ids=list(range(N_CORES)), **spmd_kwargs
        )
    out_re = np.concatenate([res.results[c]["out_re"] for c in range(N_CORES)], axis=0)
    out_im = np.concatenate([res.results[c]["out_im"] for c in range(N_CORES)], axis=0)
    return (out_re, out_im), res


def kernel(state_re, state_im, theta):
    (out_re, out_im), _ = _run(state_re, state_im, theta)
    return out_re, out_im
